# revision 1
# baseline (speedup 1.0000x reference)
"""Causal self-attention Trainium2 kernel.

Problem: B=4, T=2048, D=1024, H=16 heads (hd=64).
Sharding: 8 cores; core c -> batch c//2, heads (c%2)*8 .. +8.
Each core computes a partial output projection (its 512 rows of w_proj);
host sums the two partials per batch and adds b_proj.

Layout strategy (per core):
  - x^T [D, T] streamed in fp32, consumed as float32r (full-rate matmuls
    at near-fp32 precision for the QKV projections; host pre-transposed).
  - Q^T, K^T computed as [512, 2048] (head-dim on partitions) via
    W-stationary matmuls: out = W_chunk.T @ x^T, stored bf16.
  - V computed in natural [T, 512] layout (x^T-stationary), stored per-head
    augmented with a ones column -> [128k, head, 65], so the P@V matmul
    accumulates softmax denominators for free in row 64.
  - Scores computed transposed: S^T[k, q] = (K^T_chunk).T @ Q^T, causal
    blocks only; exp on ScalarE straight out of PSUM (no max subtraction --
    scaled scores are ~N(0,1), max << 88); triangular mask multiply only on
    diagonal 128-blocks.
  - P@V with V_aug stationary: out^T[65, q] accumulated over k-chunks in
    PSUM. Row 64 = sum of exp. Normalize with DVE reciprocal + GpSimd
    partition_broadcast; result written as A^T [512, 2048] bf16 which is
    exactly the lhsT needed for the output projection.
"""

import sys

for _p in ("/opt/trn_rl_repo",):
    if _p not in sys.path:
        sys.path.insert(0, _p)

import numpy as np
import ml_dtypes

import concourse.bass as bass
import concourse.mybir as mybir
import concourse.tile as tile
from concourse import bacc
from concourse.bass_utils import run_bass_kernel_spmd

BF16 = ml_dtypes.bfloat16

B, T, D = 4, 2048, 1024
H, HD = 16, 64
NCORES = 8
HPC = 8                  # heads per core
GCOLS = HPC * HD         # 512 columns of qkv per core per q/k/v
P = 128
NDC = D // P             # 8 contraction chunks of 128
NTT = T // P             # 16 t-tiles of 128
NQC = T // 512           # 4 q-chunks of 512
NMC = GCOLS // P         # 4 M-chunks per Q^T / K^T


def build_nc(trace_sim: bool = False):
    f32 = mybir.dt.float32
    f32r = mybir.dt.float32r
    bf16 = mybir.dt.bfloat16

    nc = bacc.Bacc("TRN2", target_bir_lowering=False, debug=False,
                   num_devices=NCORES)

    xT_d = nc.dram_tensor("xT", [D, T], f32r, kind="ExternalInput")
    wqk_d = nc.dram_tensor("wqk", [D, 2 * GCOLS], f32r, kind="ExternalInput")
    wv_d = nc.dram_tensor("wv", [D, GCOLS], f32r, kind="ExternalInput")
    wp_d = nc.dram_tensor("wp", [GCOLS, D], bf16, kind="ExternalInput")
    bqk_d = nc.dram_tensor("bqk", [P, 2 * NMC], f32, kind="ExternalInput")
    bv_d = nc.dram_tensor("bv", [GCOLS], f32, kind="ExternalInput")
    tri_d = nc.dram_tensor("tri", [P, P], bf16, kind="ExternalInput")
    out_d = nc.dram_tensor("outp", [T, D], f32, kind="ExternalOutput")

    with tile.TileContext(nc, trace_sim=trace_sim) as tc:
        with (
            tc.tile_pool(name="consts", bufs=1) as consts,
            tc.tile_pool(name="weights", bufs=1) as weights,
            tc.tile_pool(name="acts", bufs=1) as acts,
            tc.tile_pool(name="pt", bufs=3) as ptp,
            tc.tile_pool(name="norm", bufs=2) as normp,
            tc.tile_pool(name="outs", bufs=3) as outsp,
            tc.tile_pool(name="ps_mm", bufs=2, space="PSUM") as ps_mm,
            tc.tile_pool(name="ps_st", bufs=2, space="PSUM") as ps_st,
            tc.tile_pool(name="ps_o", bufs=2, space="PSUM") as ps_o,
        ):
            # First bytes on the wire: x^T piece 0 plus mch-0's K/Q weight
            # columns (K0 = cols 512:640, Q0 = cols 0:128) -- that is all the
            # first score matmuls need. wv follows for the P@V drain;
            # small constants ride after the critical stream.
            wv_sb = weights.tile([P, NDC, GCOLS], f32r)
            xT_sb = acts.tile([P, NDC, T], f32r)
            wqk_sb = weights.tile([P, NDC, 2 * GCOLS], f32r)
            for dc in range(NDC):
                nc.sync.dma_start(
                    xT_sb[:, dc, 0:512], xT_d[dc * P:(dc + 1) * P, 0:512])
                nc.sync.dma_start(wqk_sb[:, dc, GCOLS:GCOLS + P],
                                  wqk_d[dc * P:(dc + 1) * P, GCOLS:GCOLS + P])
                nc.sync.dma_start(wqk_sb[:, dc, 0:P],
                                  wqk_d[dc * P:(dc + 1) * P, 0:P])
            bqk_sb = consts.tile([P, 2 * NMC], f32)
            nc.sync.dma_start(bqk_sb[:], bqk_d.ap())
            tri_sb = consts.tile([P, P], bf16)
            nc.sync.dma_start(tri_sb[:], tri_d.ap())
            for dc in range(NDC):
                nc.sync.dma_start(wv_sb[:, dc, :], wv_d[dc * P:(dc + 1) * P, :])
            # b_v replicated to all partitions (varies along free dim)
            bv_rep = consts.tile([P, GCOLS], f32)
            bv_ap = bv_d.ap()
            nc.gpsimd.dma_start(
                bv_rep[:],
                bass.AP(tensor=bv_ap.tensor, offset=bv_ap.offset,
                        ap=[[0, P]] + list(bv_ap.ap)),
            )
            for cp in range(1, NQC):
                for dc in range(NDC):
                    nc.sync.dma_start(
                        xT_sb[:, dc, cp * 512:(cp + 1) * 512],
                        xT_d[dc * P:(dc + 1) * P, cp * 512:(cp + 1) * 512])
            for dc in range(NDC):
                nc.sync.dma_start(wqk_sb[:, dc, P:GCOLS],
                                  wqk_d[dc * P:(dc + 1) * P, P:GCOLS])
                nc.sync.dma_start(wqk_sb[:, dc, GCOLS + P:],
                                  wqk_d[dc * P:(dc + 1) * P, GCOLS + P:])
            wp_sb = weights.tile([P, NMC, D], bf16)
            for hc in range(NMC):
                nc.sync.dma_start(wp_sb[:, hc, :], wp_d[hc * P:(hc + 1) * P, :])

            # ---------------- phases 1+2 interleaved ----------------
            # warm the ScalarE Exp table during the startup DMA window so
            # the first attention block doesn't pay the table load
            warm = consts.tile([1, 1], f32)
            nc.vector.memset(warm[:], 0.0)
            nc.scalar.activation(warm[:], warm[:],
                                 mybir.ActivationFunctionType.Exp)

            # V natural + ones column: [128, tt, head, 65]
            V_sb = acts.tile([P, NTT, HPC, HD + 1], bf16)
            nc.vector.memset(V_sb[:, :, :, HD], 1.0)

            def project_v(tts):
                for tt in tts:
                    pv = ps_mm.tile([P, 512], f32, tag="mm")
                    for dc in range(NDC):
                        nc.tensor.matmul(
                            pv[:],
                            xT_sb[:, dc, tt * P:(tt + 1) * P],
                            wv_sb[:, dc, :],
                            start=(dc == 0), stop=(dc == NDC - 1),
                        )
                    nc.vector.tensor_tensor(
                        V_sb[:, tt, :, 0:HD],
                        pv[:].rearrange("p (h d) -> p h d", h=HPC),
                        bv_rep[:].rearrange("p (h d) -> p h d", h=HPC),
                        mybir.AluOpType.add,
                    )

            # Q^T / K^T / A^T: [512, T] each, stored as [128, chunk, T].
            QT_sb = acts.tile([P, NMC, T], bf16)
            KT_sb = acts.tile([P, NMC, T], bf16)
            AT_sb = acts.tile([P, NMC, T], bf16)

            def project_qk(m, tc4s=range(NQC)):
                for tc4 in tc4s:
                    pq = ps_mm.tile([P, 512], f32, tag="mm")
                    for dc in range(NDC):
                        nc.tensor.matmul(
                            pq[:],
                            wqk_sb[:, dc, m * P:(m + 1) * P],
                            xT_sb[:, dc, tc4 * 512:(tc4 + 1) * 512],
                            start=(dc == 0), stop=(dc == NDC - 1),
                        )
                    dst = (QT_sb if m < NMC else KT_sb)
                    nc.vector.tensor_scalar_add(
                        dst[:, m % NMC, tc4 * 512:(tc4 + 1) * 512],
                        pq[:], bqk_sb[:, m:m + 1],
                    )

            def project_out(tts):
                for tt in tts:
                    for ncol in range(2):
                        pp = ps_mm.tile([P, 512], f32, tag="mm")
                        for hc in range(NMC):
                            nc.tensor.matmul(
                                pp[:],
                                AT_sb[:, hc, tt * P:(tt + 1) * P],
                                wp_sb[:, hc, ncol * 512:(ncol + 1) * 512],
                                start=(hc == 0), stop=(hc == NMC - 1),
                            )
                        ot = outsp.tile([P, 512], f32, tag="ot")
                        nc.vector.tensor_copy(ot[:], pp[:])
                        nc.sync.dma_start(
                            out_d[tt * P:(tt + 1) * P,
                                  ncol * 512:(ncol + 1) * 512],
                            ot[:],
                        )

            # Per 128-chunk: project K then Q, then both heads' attention.
            # The head pair sits at partitions 0-63 / 64-127, so the two
            # K=64 score matmuls auto-derive tile_position (0,0)/(64,0)
            # and can run concurrently on the two PE array row-halves.
            # V projection is smeared across mch 0's qc blocks (only V
            # k-tiles <= 4qc+3 are needed there); the output projection is
            # smeared across mch 3's qc blocks (proj of t-range qc needs
            # every head's qc block done).
            project_qk(NMC, [0])        # K chunk 0, columns for qc 0
            project_qk(0, [0])          # Q chunk 0, columns for qc 0
            project_v(range(0, 4))
            project_qk(NMC, [1, 2, 3])
            project_qk(0, [1, 2, 3])
            for mch in range(NMC):
                # last chunk: biggest qc block first so its projection
                # groups overlap the remaining attention blocks
                qc_order = range(NQC - 1, -1, -1) if mch == NMC - 1 \
                    else range(NQC)
                for qc in qc_order:
                    po = [ps_o.tile([HD + 1, 512], f32, tag="po",
                                    name=f"po{j}")
                          for j in range(2)]
                    nki = 4 * qc + 4
                    for ki in range(nki):
                        off = max(0, ki - 4 * qc) * P
                        # head-pair S^T into one 2-bank psum tile; one exp
                        # instruction covers both heads (halves the per-op
                        # PSUM-access overhead on ScalarE).
                        pshat = ps_st.tile([P, 2, 512], f32, tag="st")
                        pts = ptp.tile([P, 2, 512], bf16, tag="pt")
                        for j in range(2):
                            part = j * 64
                            nc.tensor.matmul(
                                pshat[:, j, off:512],
                                KT_sb[part:part + 64, mch,
                                      ki * P:(ki + 1) * P],
                                QT_sb[part:part + 64, mch,
                                      qc * 512 + off:(qc + 1) * 512],
                                start=True, stop=True,
                            )
                        nc.scalar.activation(
                            pts[:, :, off:512], pshat[:, :, off:512],
                            mybir.ActivationFunctionType.Exp,
                            scale=0.125,
                        )
                        if ki >= 4 * qc:
                            # diagonal block: zero out q < k entries
                            for j in range(2):
                                nc.vector.tensor_tensor(
                                    pts[:, j, off:off + P],
                                    pts[:, j, off:off + P],
                                    tri_sb[:], mybir.AluOpType.mult,
                                )
                        for j in range(2):
                            nc.tensor.matmul(
                                po[j][:, off:512],
                                V_sb[:, ki, 2 * mch + j, :],
                                pts[:, j, off:512],
                                start=(ki == 0), stop=(ki == nki - 1),
                            )
                    # normalize: row 64 of po is the softmax denominator.
                    # Copy the unnormalized block out first so the PSUM
                    # slot frees after two quick DVE ops; the broadcast +
                    # in-place multiply run off the critical path.
                    for j in range(2):
                        part = j * 64
                        rs = normp.tile([1, 512], f32, tag="rs")
                        nc.vector.reciprocal(rs[:], po[j][HD:HD + 1, :])
                        rep = normp.tile([64, 512], f32, tag="rep")
                        nc.gpsimd.partition_broadcast(rep[:], rs[0:1, :])
                        nc.vector.tensor_tensor(
                            AT_sb[part:part + 64, mch,
                                  qc * 512:(qc + 1) * 512],
                            po[j][0:HD, :], rep[:], mybir.AluOpType.mult,
                        )
                    if mch == 0 and qc < NQC - 1:
                        project_v(range(4 * (qc + 1), 4 * (qc + 2)))
                    if mch < NMC - 1:
                        # pipeline next chunk's K/Q projection (2 of the 8
                        # 512-col groups per qc block)
                        m_next = [NMC + mch + 1, mch + 1][qc // 2]
                        project_qk(m_next, [2 * qc % 4, 2 * qc % 4 + 1])
                    else:
                        project_out(range(4 * qc, 4 * (qc + 1)))

    nc.compile()
    return nc


def host_inputs(x, w_qkv, b_qkv):
    """Per-core input maps. Core c -> batch c//2, head group c%2."""
    x = np.asarray(x, np.float32)
    w_qkv = np.asarray(w_qkv, np.float32)
    b_qkv = np.asarray(b_qkv, np.float32)
    tri = (np.arange(P)[None, :] >= np.arange(P)[:, None]).astype(BF16)
    in_maps = []
    for c in range(NCORES):
        b, g = c // 2, c % 2
        cs = slice(g * GCOLS, (g + 1) * GCOLS)
        xT = np.ascontiguousarray(x[b].T).astype(np.float32)
        wqk = np.concatenate(
            [w_qkv[:, cs], w_qkv[:, D + g * GCOLS: D + (g + 1) * GCOLS]],
            axis=1).astype(np.float32)
        wv = np.ascontiguousarray(
            w_qkv[:, 2 * D + g * GCOLS: 2 * D + (g + 1) * GCOLS]).astype(np.float32)
        bq = b_qkv[cs].reshape(NMC, P).T
        bk = b_qkv[D + g * GCOLS: D + (g + 1) * GCOLS].reshape(NMC, P).T
        bqk = np.ascontiguousarray(
            np.concatenate([bq, bk], axis=1)).astype(np.float32)
        bv = np.ascontiguousarray(
            b_qkv[2 * D + g * GCOLS: 2 * D + (g + 1) * GCOLS]).astype(np.float32)
        in_maps.append({
            "xT": xT, "wqk": wqk, "wv": wv,
            "wp": None,  # filled by caller (needs w_proj)
            "bqk": bqk, "bv": bv, "tri": tri,
        })
    return in_maps


def full_in_maps(x, w_qkv, b_qkv, w_proj):
    w_proj = np.asarray(w_proj, np.float32)
    in_maps = host_inputs(x, w_qkv, b_qkv)
    for c in range(NCORES):
        g = c % 2
        in_maps[c]["wp"] = np.ascontiguousarray(
            w_proj[g * GCOLS:(g + 1) * GCOLS, :]).astype(BF16)
    return in_maps


def gather(results, b_proj):
    out = np.zeros((B, T, D), np.float32)
    for c in range(NCORES):
        out[c // 2] += results[c]["outp"]
    out += np.asarray(b_proj, np.float32)[None, None, :]
    return out


_NC_CACHE = None


def kernel(x, w_qkv, b_qkv, w_proj, b_proj):
    global _NC_CACHE
    if _NC_CACHE is None:
        _NC_CACHE = build_nc()
    in_maps = full_in_maps(x, w_qkv, b_qkv, w_proj)
    res = run_bass_kernel_spmd(_NC_CACHE, in_maps, core_ids=list(range(NCORES)))
    return gather(res.results, b_proj)


if __name__ == "__main__":
    rng = np.random.default_rng(0)
    x = rng.standard_normal((B, T, D), dtype=np.float32)
    w_qkv = rng.standard_normal((D, 3 * D), dtype=np.float32) / np.sqrt(D)
    b_qkv = np.zeros(3 * D, np.float32)
    w_proj = rng.standard_normal((D, D), dtype=np.float32) / np.sqrt(D)
    b_proj = np.zeros(D, np.float32)
    out = kernel(x, w_qkv, b_qkv, w_proj, b_proj)
    print(out.shape, out.dtype)



# revision 41
# speedup vs baseline: 1.1089x; 1.1089x over previous
"""Causal self-attention Trainium2 kernel.

Problem: B=4, T=2048, D=1024, H=16 heads (hd=64).
Sharding: 8 cores; core c -> batch c//2, heads (c%2)*8 .. +8.
Each core computes a partial output projection (its 512 rows of w_proj);
host sums the two partials per batch and adds b_proj.

Layout strategy (per core):
  - x^T [D, T] streamed in fp32, consumed as float32r (full-rate matmuls
    at near-fp32 precision for the QKV projections; host pre-transposed).
  - Q^T, K^T computed as [512, 2048] (head-dim on partitions) via
    W-stationary matmuls: out = W_chunk.T @ x^T, stored bf16.
  - V computed in natural [T, 512] layout (x^T-stationary), stored per-head
    augmented with a ones column -> [128k, head, 65], so the P@V matmul
    accumulates softmax denominators for free in row 64.
  - Scores computed transposed: S^T[k, q] = (K^T_chunk).T @ Q^T, causal
    blocks only; exp on ScalarE straight out of PSUM (no max subtraction --
    scaled scores are ~N(0,1), max << 88); triangular mask multiply only on
    diagonal 128-blocks.
  - P@V with V_aug stationary: out^T[65, q] accumulated over k-chunks in
    PSUM. Row 64 = sum of exp. Normalize with DVE reciprocal + GpSimd
    partition_broadcast; result written as A^T [512, 2048] bf16 which is
    exactly the lhsT needed for the output projection.

Schedule strategy (this revision):
  - All input DMAs are batched into few multi-level-AP copies and split
    across the SP and Activation HWDGE queues (plus Pool SWDGE for small
    constants) so dispatch overhead (~650ns/copy/queue) never gates the
    startup.
  - All projection work (V, K/Q chunks, output projection) is expressed as
    generators yielding one matmul at a time.  A deadline-ordered filler
    queue is pumped a few matmuls per attention k-step, so the Tensor
    engine always has ready work while the Exp activation (which otherwise
    paces the attention inner loop at ~1.04us per 128-k-block) runs ahead.
  - The P@V matmul for block ki is emitted one iteration late (software
    pipelining) so it never blocks the in-order PE queue waiting on exp.
"""

import sys
from collections import deque

for _p in ("/opt/trn_rl_repo",):
    if _p not in sys.path:
        sys.path.insert(0, _p)

import numpy as np
import ml_dtypes

import concourse.bass as bass
import concourse.mybir as mybir
import concourse.tile as tile
from concourse import bacc
from concourse.bass_utils import run_bass_kernel_spmd

BF16 = ml_dtypes.bfloat16

B, T, D = 4, 2048, 1024
H, HD = 16, 64
NCORES = 8
HPC = 8                  # heads per core
GCOLS = HPC * HD         # 512 columns of qkv per core per q/k/v
P = 128
NDC = D // P             # 8 contraction chunks of 128
NTT = T // P             # 16 t-tiles of 128
NQC = T // 512           # 4 q-chunks of 512
NMC = GCOLS // P         # 4 M-chunks per Q^T / K^T


def _ap3(dram, off, part_stride, nparts, mid_stride, nmid, inner):
    """3-level DRAM access pattern: [partition, mid, contiguous-inner]."""
    a = dram.ap()
    return bass.AP(tensor=a.tensor, offset=a.offset + off,
                   ap=[[part_stride, nparts], [mid_stride, nmid], [1, inner]])


def build_nc(trace_sim: bool = False):
    f32 = mybir.dt.float32
    f32r = mybir.dt.float32r
    bf16 = mybir.dt.bfloat16

    nc = bacc.Bacc("TRN2", target_bir_lowering=False, debug=False,
                   num_devices=NCORES)

    xT_d = nc.dram_tensor("xT", [D, T], bf16, kind="ExternalInput")
    # wkq: interleaved 128-col blocks [K0 Q0 K1 Q1 K2 Q2 K3 Q3]
    wkq_d = nc.dram_tensor("wkq", [D, 2 * GCOLS], bf16, kind="ExternalInput")
    wv_d = nc.dram_tensor("wv", [D, GCOLS], bf16, kind="ExternalInput")
    wp_d = nc.dram_tensor("wp", [GCOLS, D], bf16, kind="ExternalInput")
    # bqk: col 2m = bias for K_m block, col 2m+1 = bias for Q_m block
    bqk_d = nc.dram_tensor("bqk", [P, 2 * NMC], f32, kind="ExternalInput")
    bv_d = nc.dram_tensor("bv", [GCOLS], f32, kind="ExternalInput")
    tri_d = nc.dram_tensor("tri", [P, P], bf16, kind="ExternalInput")
    out_d = nc.dram_tensor("outp", [T, D], bf16, kind="ExternalOutput")

    with tile.TileContext(nc, trace_sim=trace_sim) as tc:
        with (
            tc.tile_pool(name="consts", bufs=1) as consts,
            tc.tile_pool(name="weights", bufs=1) as weights,
            tc.tile_pool(name="acts", bufs=1) as acts,
            tc.tile_pool(name="pt", bufs=3) as ptp,
            tc.tile_pool(name="norm", bufs=2) as normp,
            tc.tile_pool(name="outs", bufs=3) as outsp,
            tc.tile_pool(name="ps_mm", bufs=2, space="PSUM") as ps_mm,
            tc.tile_pool(name="ps_st", bufs=2, space="PSUM") as ps_st,
            tc.tile_pool(name="ps_o", bufs=2, space="PSUM") as ps_o,
        ):
            wv_sb = weights.tile([P, NDC, GCOLS], bf16)
            xT_sb = acts.tile([P, NDC, T], bf16)
            wkq_sb = weights.tile([P, NDC, 2 * GCOLS], bf16)
            wp_sb = weights.tile([P, NMC, D], bf16)

            # ---- Single SP HWDGE stream in priority order: the HWDGE FIFO
            # is shared, so one in-order queue gives exact control of what
            # bytes land first.  V work (wv + x t-slices) unlocks the most
            # early PE work per byte; wkq m0 gates the first attention
            # block; bulk x and later wkq/wp chunks follow.
            def dma_x(c0, c1):
                nc.sync.dma_start(
                    xT_sb[:, :, c0:c1],
                    _ap3(xT_d, c0, T, P, P * T, NDC, c1 - c0))

            def dma_wkq(m):
                nc.sync.dma_start(
                    wkq_sb[:, :, 256 * m:256 * (m + 1)],
                    _ap3(wkq_d, 256 * m, 2 * GCOLS, P,
                         P * 2 * GCOLS, NDC, 256))

            def dma_x_dc(d0, d1, c0, c1):
                nc.sync.dma_start(
                    xT_sb[:, d0:d1, c0:c1],
                    _ap3(xT_d, d0 * P * T + c0, T, P, P * T, d1 - d0,
                         c1 - c0))

            def dma_wv(d0, d1):
                nc.sync.dma_start(
                    wv_sb[:, d0:d1, :],
                    _ap3(wv_d, d0 * P * GCOLS, GCOLS, P, P * GCOLS,
                         d1 - d0, GCOLS))

            # bf16: t-slices must be >=256 cols to keep 512B-contiguous
            # descriptors (full DMA rate)
            dma_x_dc(0, 4, 0, 256)
            dma_wv(0, 2)
            dma_x_dc(4, 8, 0, 256)
            dma_wv(2, 4)
            dma_x(256, 512)
            dma_wv(4, 6)
            dma_wv(6, 8)
            dma_wkq(0)
            dma_x(512, 1024)
            dma_x(1024, 1536)
            dma_x(1536, 2048)
            for m in range(1, NMC):
                dma_wkq(m)
            nc.sync.dma_start(wp_sb[:, :, :],
                              _ap3(wp_d, 0, D, P, P * D, NMC, D))
            # ---- Pool SWDGE: small constants ----
            bqk_sb = consts.tile([P, 2 * NMC], f32)
            nc.gpsimd.dma_start(bqk_sb[:], bqk_d.ap())
            tri_sb = consts.tile([P, P], bf16)
            nc.gpsimd.dma_start(tri_sb[:], tri_d.ap())
            bv_rep = consts.tile([P, GCOLS], f32)
            bv_ap = bv_d.ap()
            nc.gpsimd.dma_start(
                bv_rep[:],
                bass.AP(tensor=bv_ap.tensor, offset=bv_ap.offset,
                        ap=[[0, P]] + list(bv_ap.ap)),
            )

            # warm the ScalarE Exp table during the startup DMA window
            warm = consts.tile([1, 1], f32)
            nc.vector.memset(warm[:], 0.0)
            nc.scalar.activation(warm[:], warm[:],
                                 mybir.ActivationFunctionType.Exp)

            # V natural + ones column: [128, tt, head, 65]
            V_sb = acts.tile([P, NTT, HPC, HD + 1], bf16)
            nc.vector.memset(V_sb[:, :, :, HD], 1.0)

            QT_sb = acts.tile([P, NMC, T], bf16)
            KT_sb = acts.tile([P, NMC, T], bf16)
            AT_sb = acts.tile([P, NMC, T], bf16)

            # ---------------- filler generators ----------------
            def gen_v(tt):
                pv = ps_mm.tile([P, 512], f32, tag="mm", name=f"pv{tt}")
                for dc in range(NDC):
                    nc.tensor.matmul(
                        pv[:],
                        xT_sb[:, dc, tt * P:(tt + 1) * P],
                        wv_sb[:, dc, :],
                        start=(dc == 0), stop=(dc == NDC - 1),
                    )
                    if dc < NDC - 1:
                        yield
                # NOTE: PSUM readers must be PE/DVE/Act (GPSIMD cannot
                # access PSUM on hardware)
                nc.vector.tensor_tensor(
                    V_sb[:, tt, :, 0:HD],
                    pv[:].rearrange("p (h d) -> p h d", h=HPC),
                    bv_rep[:].rearrange("p (h d) -> p h d", h=HPC),
                    mybir.AluOpType.add,
                )

            def gen_kq(is_q, m, tc4):
                col = 256 * m + (128 if is_q else 0)
                pq = ps_mm.tile([P, 512], f32, tag="mm",
                                name=f"p{'q' if is_q else 'k'}{m}_{tc4}")
                for dc in range(NDC):
                    nc.tensor.matmul(
                        pq[:],
                        wkq_sb[:, dc, col:col + P],
                        xT_sb[:, dc, tc4 * 512:(tc4 + 1) * 512],
                        start=(dc == 0), stop=(dc == NDC - 1),
                    )
                    if dc < NDC - 1:
                        yield
                dst = QT_sb if is_q else KT_sb
                bcol = 2 * m + (1 if is_q else 0)
                nc.vector.tensor_scalar_add(
                    dst[:, m, tc4 * 512:(tc4 + 1) * 512],
                    pq[:], bqk_sb[:, bcol:bcol + 1],
                )

            def gen_out(tt, ncol, pool_tag=None, tail=False):
                pool, tag = pool_tag or (ps_mm, "mm")
                pp = pool.tile([P, 512], f32, tag=tag,
                               name=f"pp{tt}_{ncol}")
                for hc in range(NMC):
                    nc.tensor.matmul(
                        pp[:],
                        AT_sb[:, hc, tt * P:(tt + 1) * P],
                        wp_sb[:, hc, ncol * 512:(ncol + 1) * 512],
                        start=(hc == 0), stop=(hc == NMC - 1),
                    )
                    if hc < NMC - 1:
                        yield
                ot = outsp.tile([P, 512], bf16, tag="ot", bufs=7)
                # PSUM->SBUF copy: DVE normally; in the tail the exp stream
                # is over, so Act's activation-Copy path halves the ladder
                if tail and ncol == 1:
                    nc.scalar.activation(ot[:], pp[:],
                                         mybir.ActivationFunctionType.Copy)
                else:
                    nc.vector.tensor_copy(ot[:], pp[:])
                # tail groups run after the exp stream is over, so the Act
                # HWDGE queue is free to halve the dispatch ladder
                deng = nc.scalar if (tail and ncol == 1) else nc.sync
                deng.dma_start(
                    out_d[tt * P:(tt + 1) * P,
                          ncol * 512:(ncol + 1) * 512],
                    ot[:],
                )

            fillers = deque()   # (deadline, generator)

            def pump(n):
                while n > 0 and fillers:
                    try:
                        next(fillers[0][1])
                    except StopIteration:
                        fillers.popleft()
                    n -= 1

            def drain_until(deadline):
                while fillers and fillers[0][0] <= deadline:
                    for _ in fillers.popleft()[1]:
                        pass

            def drain_rr():
                # round-robin across remaining generators so independent
                # matmuls (early hc chunks of each out-proj group) are
                # emitted ahead of ones gated on the final normalize
                while fillers:
                    _, g = fillers.popleft()
                    try:
                        next(g)
                    except StopIteration:
                        continue
                    fillers.append((None, g))

            def run_gen(g):
                for _ in g:
                    pass

            # ---------------- startup compute ----------------
            for tt in range(4):
                run_gen(gen_v(tt))
            run_gen(gen_kq(False, 0, 0))
            run_gen(gen_kq(True, 0, 0))

            # deadline-ordered filler queue (deadline = (mch, qc) at whose
            # start the group's output is first consumed)
            for m in range(NMC):
                for qc in range(NQC):
                    if m == 0 and qc == 0:
                        continue
                    fillers.append(((m, qc), gen_kq(False, m, qc)))
                    fillers.append(((m, qc), gen_kq(True, m, qc)))
                    if m == 0:
                        for tt in range(4 * qc, 4 * qc + 4):
                            fillers.append(((m, qc), gen_v(tt)))

            # ---------------- attention main loop ----------------
            for mch in range(NMC):
                for qc in range(NQC):
                    drain_until((mch, qc))
                    po = [ps_o.tile([HD + 1, 512], f32, tag="po",
                                    name=f"po{mch}_{qc}_{j}")
                          for j in range(2)]
                    nki = 4 * qc + 4
                    prev = None
                    for ki in range(nki):
                        off = max(0, ki - 4 * qc) * P
                        pshat = ps_st.tile([P, 2, 512], f32, tag="st")
                        pts = ptp.tile([P, 2, 512], bf16, tag="pt")
                        for j in range(2):
                            part = j * 64
                            nc.tensor.matmul(
                                pshat[:, j, off:512],
                                KT_sb[part:part + 64, mch,
                                      ki * P:(ki + 1) * P],
                                QT_sb[part:part + 64, mch,
                                      qc * 512 + off:(qc + 1) * 512],
                                start=True, stop=True,
                            )
                        nc.scalar.activation(
                            pts[:, :, off:512], pshat[:, :, off:512],
                            mybir.ActivationFunctionType.Exp,
                            scale=0.125,
                        )
                        if ki >= 4 * qc:
                            # diagonal block: zero out q < k entries
                            for j in range(2):
                                nc.vector.tensor_tensor(
                                    pts[:, j, off:off + P],
                                    pts[:, j, off:off + P],
                                    tri_sb[:], mybir.AluOpType.mult,
                                )
                        pump(3 if mch == NMC - 1 else 2)
                        if prev is not None:
                            poff, ppts = prev
                            for j in range(2):
                                nc.tensor.matmul(
                                    po[j][:, poff:512],
                                    V_sb[:, ki - 1, 2 * mch + j, :],
                                    ppts[:, j, poff:512],
                                    start=(ki - 1 == 0), stop=False,
                                )
                        prev = (off, pts)
                    poff, ppts = prev
                    for j in range(2):
                        nc.tensor.matmul(
                            po[j][:, poff:512],
                            V_sb[:, nki - 1, 2 * mch + j, :],
                            ppts[:, j, poff:512],
                            start=(nki == 1), stop=True,
                        )
                    # normalize: row 64 of po is the softmax denominator
                    for j in range(2):
                        part = j * 64
                        rs = normp.tile([1, 512], f32, tag="rs")
                        nc.vector.reciprocal(rs[:], po[j][HD:HD + 1, :])
                        rep = normp.tile([64, 512], f32, tag="rep")
                        nc.gpsimd.partition_broadcast(rep[:], rs[0:1, :])
                        nc.vector.tensor_tensor(
                            AT_sb[part:part + 64, mch,
                                  qc * 512:(qc + 1) * 512],
                            po[j][0:HD, :], rep[:], mybir.AluOpType.mult,
                        )
                    if mch == NMC - 1:
                        if qc < NQC - 1:
                            for tt in range(4 * qc, 4 * qc + 4):
                                for ncol in range(2):
                                    fillers.append(((9, qc),
                                                    gen_out(tt, ncol)))
                        else:
                            # last qc: spread groups across the (then idle)
                            # attention PSUM pools for tail concurrency; the
                            # mm-slot groups go last since their slots free
                            # last (gated on pumped groups' copies)
                            t0 = 4 * qc
                            plan = [
                                ((t0 + 0, 0), (ps_st, "st")),
                                ((t0 + 0, 1), (ps_st, "st")),
                                ((t0 + 1, 0), (ps_o, "po")),
                                ((t0 + 1, 1), (ps_o, "po")),
                                ((t0 + 2, 0), (ps_st, "st")),
                                ((t0 + 2, 1), (ps_o, "po")),
                                ((t0 + 3, 0), None),
                                ((t0 + 3, 1), None),
                            ]
                            for (tt, ncol), pt in plan:
                                fillers.append(((9, qc),
                                                gen_out(tt, ncol, pt,
                                                        tail=True)))
            # drain remaining fillers (last qc's output projection)
            drain_rr()

    nc.compile()
    return nc


def host_inputs(x, w_qkv, b_qkv):
    """Per-core input maps. Core c -> batch c//2, head group c%2."""
    x = np.asarray(x, np.float32)
    w_qkv = np.asarray(w_qkv, np.float32)
    b_qkv = np.asarray(b_qkv, np.float32)
    tri = (np.arange(P)[None, :] >= np.arange(P)[:, None]).astype(BF16)
    in_maps = []
    for c in range(NCORES):
        b, g = c // 2, c % 2
        xT = np.ascontiguousarray(x[b].T).astype(BF16)
        # interleaved [K_m | Q_m] 128-col pairs
        wkq = np.empty((D, 2 * GCOLS), np.float32)
        bqk = np.empty((P, 2 * NMC), np.float32)
        for m in range(NMC):
            qs = g * GCOLS + m * P
            ks = D + g * GCOLS + m * P
            wkq[:, 256 * m:256 * m + P] = w_qkv[:, ks:ks + P]
            wkq[:, 256 * m + P:256 * (m + 1)] = w_qkv[:, qs:qs + P]
            bqk[:, 2 * m] = b_qkv[ks:ks + P]
            bqk[:, 2 * m + 1] = b_qkv[qs:qs + P]
        wkq = wkq.astype(BF16)
        wv = np.ascontiguousarray(
            w_qkv[:, 2 * D + g * GCOLS: 2 * D + (g + 1) * GCOLS]).astype(BF16)
        bv = np.ascontiguousarray(
            b_qkv[2 * D + g * GCOLS: 2 * D + (g + 1) * GCOLS]).astype(np.float32)
        in_maps.append({
            "xT": xT, "wkq": wkq, "wv": wv,
            "wp": None,  # filled by caller (needs w_proj)
            "bqk": bqk, "bv": bv, "tri": tri,
        })
    return in_maps


def full_in_maps(x, w_qkv, b_qkv, w_proj):
    w_proj = np.asarray(w_proj, np.float32)
    in_maps = host_inputs(x, w_qkv, b_qkv)
    for c in range(NCORES):
        g = c % 2
        in_maps[c]["wp"] = np.ascontiguousarray(
            w_proj[g * GCOLS:(g + 1) * GCOLS, :]).astype(BF16)
    return in_maps


def gather(results, b_proj):
    out = np.zeros((B, T, D), np.float32)
    for c in range(NCORES):
        out[c // 2] += results[c]["outp"].astype(np.float32)
    out += np.asarray(b_proj, np.float32)[None, None, :]
    return out


_NC_CACHE = None


def kernel(x, w_qkv, b_qkv, w_proj, b_proj):
    global _NC_CACHE
    if _NC_CACHE is None:
        _NC_CACHE = build_nc()
    in_maps = full_in_maps(x, w_qkv, b_qkv, w_proj)
    res = run_bass_kernel_spmd(_NC_CACHE, in_maps, core_ids=list(range(NCORES)))
    return gather(res.results, b_proj)


if __name__ == "__main__":
    rng = np.random.default_rng(0)
    x = rng.standard_normal((B, T, D), dtype=np.float32)
    w_qkv = rng.standard_normal((D, 3 * D), dtype=np.float32) / np.sqrt(D)
    b_qkv = np.zeros(3 * D, np.float32)
    w_proj = rng.standard_normal((D, D), dtype=np.float32) / np.sqrt(D)
    b_proj = np.zeros(D, np.float32)
    out = kernel(x, w_qkv, b_qkv, w_proj, b_proj)
    print(out.shape, out.dtype)


# revision 61
# speedup vs baseline: 1.1527x; 1.0395x over previous
"""Causal self-attention Trainium2 kernel.

Problem: B=4, T=2048, D=1024, H=16 heads (hd=64).
Sharding: 8 cores; core c -> batch c//2, heads (c%2)*8 .. +8.
Each core computes a partial output projection (its 512 rows of w_proj);
host sums the two partials per batch and adds b_proj.

Layout strategy (per core):
  - x^T [D, T] streamed in fp32, consumed as float32r (full-rate matmuls
    at near-fp32 precision for the QKV projections; host pre-transposed).
  - Q^T, K^T computed as [512, 2048] (head-dim on partitions) via
    W-stationary matmuls: out = W_chunk.T @ x^T, stored bf16.
  - V computed in natural [T, 512] layout (x^T-stationary), stored per-head
    augmented with a ones column -> [128k, head, 65], so the P@V matmul
    accumulates softmax denominators for free in row 64.
  - Scores computed transposed: S^T[k, q] = (K^T_chunk).T @ Q^T, causal
    blocks only; exp on ScalarE straight out of PSUM (no max subtraction --
    scaled scores are ~N(0,1), max << 88); triangular mask multiply only on
    diagonal 128-blocks.
  - P@V with V_aug stationary: out^T[65, q] accumulated over k-chunks in
    PSUM. Row 64 = sum of exp. Normalize with DVE reciprocal + GpSimd
    partition_broadcast; result written as A^T [512, 2048] bf16 which is
    exactly the lhsT needed for the output projection.

Schedule strategy (this revision):
  - All input DMAs are batched into few multi-level-AP copies and split
    across the SP and Activation HWDGE queues (plus Pool SWDGE for small
    constants) so dispatch overhead (~650ns/copy/queue) never gates the
    startup.
  - All projection work (V, K/Q chunks, output projection) is expressed as
    generators yielding one matmul at a time.  A deadline-ordered filler
    queue is pumped a few matmuls per attention k-step, so the Tensor
    engine always has ready work while the Exp activation (which otherwise
    paces the attention inner loop at ~1.04us per 128-k-block) runs ahead.
  - The P@V matmul for block ki is emitted one iteration late (software
    pipelining) so it never blocks the in-order PE queue waiting on exp.
"""

import sys
from collections import deque

for _p in ("/opt/trn_rl_repo",):
    if _p not in sys.path:
        sys.path.insert(0, _p)

import numpy as np
import ml_dtypes

import concourse.bass as bass
import concourse.mybir as mybir
import concourse.tile as tile
from concourse import bacc
from concourse.bass_utils import run_bass_kernel_spmd

BF16 = ml_dtypes.bfloat16

B, T, D = 4, 2048, 1024
H, HD = 16, 64
NCORES = 8
HPC = 8                  # heads per core
GCOLS = HPC * HD         # 512 columns of qkv per core per q/k/v
P = 128
NDC = D // P             # 8 contraction chunks of 128
NTT = T // P             # 16 t-tiles of 128
NQC = T // 512           # 4 q-chunks of 512
NMC = GCOLS // P         # 4 M-chunks per Q^T / K^T


def _ap3(dram, off, part_stride, nparts, mid_stride, nmid, inner):
    """3-level DRAM access pattern: [partition, mid, contiguous-inner]."""
    a = dram.ap()
    return bass.AP(tensor=a.tensor, offset=a.offset + off,
                   ap=[[part_stride, nparts], [mid_stride, nmid], [1, inner]])


def build_nc(trace_sim: bool = False):
    f32 = mybir.dt.float32
    f32r = mybir.dt.float32r
    bf16 = mybir.dt.bfloat16

    nc = bacc.Bacc("TRN2", target_bir_lowering=False, debug=False,
                   num_devices=NCORES)

    xT_d = nc.dram_tensor("xT", [D, T], bf16, kind="ExternalInput")
    # wkq: interleaved 128-col blocks [K0 Q0 K1 Q1 K2 Q2 K3 Q3]
    wkq_d = nc.dram_tensor("wkq", [D, 2 * GCOLS], bf16, kind="ExternalInput")
    wv_d = nc.dram_tensor("wv", [D, GCOLS], bf16, kind="ExternalInput")
    wp_d = nc.dram_tensor("wp", [GCOLS, D], bf16, kind="ExternalInput")
    # bqk: col 2m = bias for K_m block, col 2m+1 = bias for Q_m block
    bqk_d = nc.dram_tensor("bqk", [P, 2 * NMC], f32, kind="ExternalInput")
    bv_d = nc.dram_tensor("bv", [GCOLS], f32, kind="ExternalInput")
    tri_d = nc.dram_tensor("tri", [P, P], bf16, kind="ExternalInput")
    out_d = nc.dram_tensor("outp", [T, D], bf16, kind="ExternalOutput")

    with tile.TileContext(nc, trace_sim=trace_sim) as tc:
        with (
            tc.tile_pool(name="consts", bufs=1) as consts,
            tc.tile_pool(name="weights", bufs=1) as weights,
            tc.tile_pool(name="acts", bufs=1) as acts,
            tc.tile_pool(name="pt", bufs=6) as ptp,
            tc.tile_pool(name="norm", bufs=4) as normp,
            tc.tile_pool(name="outs", bufs=3) as outsp,
            tc.tile_pool(name="ps_mm", bufs=2, space="PSUM") as ps_mm,
            tc.tile_pool(name="ps_st", bufs=2, space="PSUM") as ps_st,
            tc.tile_pool(name="ps_o", bufs=2, space="PSUM") as ps_o,
        ):
            wv_sb = weights.tile([P, NDC, GCOLS], bf16)
            xT_sb = acts.tile([P, NDC, T], bf16)
            wkq_sb = weights.tile([P, NDC, 2 * GCOLS], bf16)
            wp_sb = weights.tile([P, NMC, D], bf16)

            # ---- Single SP HWDGE stream in priority order: the HWDGE FIFO
            # is shared, so one in-order queue gives exact control of what
            # bytes land first.  V work (wv + x t-slices) unlocks the most
            # early PE work per byte; wkq m0 gates the first attention
            # block; bulk x and later wkq/wp chunks follow.
            def dma_x(c0, c1):
                nc.sync.dma_start(
                    xT_sb[:, :, c0:c1],
                    _ap3(xT_d, c0, T, P, P * T, NDC, c1 - c0))

            def dma_wkq(m):
                nc.sync.dma_start(
                    wkq_sb[:, :, 256 * m:256 * (m + 1)],
                    _ap3(wkq_d, 256 * m, 2 * GCOLS, P,
                         P * 2 * GCOLS, NDC, 256))

            def dma_x_dc(d0, d1, c0, c1):
                nc.sync.dma_start(
                    xT_sb[:, d0:d1, c0:c1],
                    _ap3(xT_d, d0 * P * T + c0, T, P, P * T, d1 - d0,
                         c1 - c0))

            def dma_wv(d0, d1):
                nc.sync.dma_start(
                    wv_sb[:, d0:d1, :],
                    _ap3(wv_d, d0 * P * GCOLS, GCOLS, P, P * GCOLS,
                         d1 - d0, GCOLS))

            # bf16: t-slices must be >=256 cols to keep 512B-contiguous
            # descriptors (full DMA rate)
            dma_x_dc(0, 4, 0, 256)
            dma_wv(0, 2)
            dma_x_dc(4, 8, 0, 256)
            dma_wv(2, 4)
            dma_x(256, 512)
            dma_wv(4, 6)
            dma_wv(6, 8)
            dma_wkq(0)
            dma_x(512, 1024)
            dma_x(1024, 1536)
            dma_x(1536, 2048)
            for m in range(1, NMC):
                dma_wkq(m)
            nc.sync.dma_start(wp_sb[:, :, :],
                              _ap3(wp_d, 0, D, P, P * D, NMC, D))
            # ---- Pool SWDGE: small constants ----
            bqk_sb = consts.tile([P, 2 * NMC], f32)
            nc.gpsimd.dma_start(bqk_sb[:], bqk_d.ap())
            tri_sb = consts.tile([P, P], bf16)
            nc.gpsimd.dma_start(tri_sb[:], tri_d.ap())
            bv_rep = consts.tile([P, GCOLS], f32)
            bv_ap = bv_d.ap()
            nc.gpsimd.dma_start(
                bv_rep[:],
                bass.AP(tensor=bv_ap.tensor, offset=bv_ap.offset,
                        ap=[[0, P]] + list(bv_ap.ap)),
            )

            # warm the ScalarE Exp table during the startup DMA window
            warm = consts.tile([1, 1], f32)
            nc.vector.memset(warm[:], 0.0)
            nc.scalar.activation(warm[:], warm[:],
                                 mybir.ActivationFunctionType.Exp)

            # V natural + ones column: [128, tt, head, 65]
            V_sb = acts.tile([P, NTT, HPC, HD + 1], bf16)
            nc.vector.memset(V_sb[:, :, :, HD], 1.0)

            QT_sb = acts.tile([P, NMC, T], bf16)
            KT_sb = acts.tile([P, NMC, T], bf16)
            AT_sb = acts.tile([P, NMC, T], bf16)
            # SBUF staging for unnormalized P@V blocks: the PSUM po slot is
            # released by a plain copy at the qc boundary; the normalize
            # (recip/broadcast/mult) is deferred into the next qc's ki loop
            # where the DVE/Pool queues are quiet
            stage_sb = acts.tile([P, 4, 2, 512], f32)

            # ---------------- filler generators ----------------
            def gen_v(tt):
                pv = ps_mm.tile([P, 512], f32, tag="mm", name=f"pv{tt}")
                for dc in range(NDC):
                    nc.tensor.matmul(
                        pv[:],
                        xT_sb[:, dc, tt * P:(tt + 1) * P],
                        wv_sb[:, dc, :],
                        start=(dc == 0), stop=(dc == NDC - 1),
                    )
                    if dc < NDC - 1:
                        yield
                # NOTE: PSUM readers must be PE/DVE/Act (GPSIMD cannot
                # access PSUM on hardware)
                nc.vector.tensor_tensor(
                    V_sb[:, tt, :, 0:HD],
                    pv[:].rearrange("p (h d) -> p h d", h=HPC),
                    bv_rep[:].rearrange("p (h d) -> p h d", h=HPC),
                    mybir.AluOpType.add,
                )

            def gen_kq(is_q, m, tc4):
                col = 256 * m + (128 if is_q else 0)
                pq = ps_mm.tile([P, 512], f32, tag="mm",
                                name=f"p{'q' if is_q else 'k'}{m}_{tc4}")
                for dc in range(NDC):
                    nc.tensor.matmul(
                        pq[:],
                        wkq_sb[:, dc, col:col + P],
                        xT_sb[:, dc, tc4 * 512:(tc4 + 1) * 512],
                        start=(dc == 0), stop=(dc == NDC - 1),
                    )
                    if dc < NDC - 1:
                        yield
                dst = QT_sb if is_q else KT_sb
                bcol = 2 * m + (1 if is_q else 0)
                nc.vector.tensor_scalar_add(
                    dst[:, m, tc4 * 512:(tc4 + 1) * 512],
                    pq[:], bqk_sb[:, bcol:bcol + 1],
                )

            def gen_out(tt, ncol, pool_tag=None, tail=False):
                pool, tag = pool_tag or (ps_mm, "mm")
                pp = pool.tile([P, 512], f32, tag=tag,
                               name=f"pp{tt}_{ncol}")
                for hc in range(NMC):
                    nc.tensor.matmul(
                        pp[:],
                        AT_sb[:, hc, tt * P:(tt + 1) * P],
                        wp_sb[:, hc, ncol * 512:(ncol + 1) * 512],
                        start=(hc == 0), stop=(hc == NMC - 1),
                    )
                    if hc < NMC - 1:
                        yield
                ot = outsp.tile([P, 512], bf16, tag="ot", bufs=7)
                # PSUM->SBUF copy: DVE normally; groups whose copies land
                # near the end (when the DVE queue is jammed with the final
                # masks/normalize but the exp stream is winding down) use
                # Act's activation-Copy path instead
                if tail:
                    nc.scalar.activation(ot[:], pp[:],
                                         mybir.ActivationFunctionType.Copy)
                else:
                    nc.vector.tensor_copy(ot[:], pp[:])
                nc.sync.dma_start(
                    out_d[tt * P:(tt + 1) * P,
                          ncol * 512:(ncol + 1) * 512],
                    ot[:],
                )

            def gen_out_pair(tt, ptA, ptB, deng):
                """Tail variant: both 512-col halves of a tt row-block, one
                combined DMA (halves the tail HWDGE ladder)."""
                poolA, tagA = ptA
                poolB, tagB = ptB
                ppA = poolA.tile([P, 512], f32, tag=tagA, name=f"ppa{tt}")
                ppB = poolB.tile([P, 512], f32, tag=tagB, name=f"ppb{tt}")
                for hc in range(NMC):
                    for pp, ncol in ((ppA, 0), (ppB, 1)):
                        nc.tensor.matmul(
                            pp[:],
                            AT_sb[:, hc, tt * P:(tt + 1) * P],
                            wp_sb[:, hc, ncol * 512:(ncol + 1) * 512],
                            start=(hc == 0), stop=(hc == NMC - 1),
                        )
                        if not (hc == NMC - 1 and ncol == 1):
                            yield
                ot2 = outsp.tile([P, 1024], bf16, tag="ot2", bufs=4)
                nc.vector.tensor_copy(ot2[:, 0:512], ppA[:])
                nc.scalar.activation(ot2[:, 512:1024], ppB[:],
                                     mybir.ActivationFunctionType.Copy)
                deng.dma_start(out_d[tt * P:(tt + 1) * P, :], ot2[:])

            fillers = deque()   # (deadline, generator)

            def pump(n):
                while n > 0 and fillers:
                    try:
                        next(fillers[0][1])
                    except StopIteration:
                        fillers.popleft()
                    n -= 1

            def drain_until(deadline):
                while fillers and fillers[0][0] <= deadline:
                    for _ in fillers.popleft()[1]:
                        pass

            def drain_rr():
                # round-robin across remaining generators so independent
                # matmuls (early hc chunks of each out-proj group) are
                # emitted ahead of ones gated on the final normalize
                while fillers:
                    _, g = fillers.popleft()
                    try:
                        next(g)
                    except StopIteration:
                        continue
                    fillers.append((None, g))

            def run_gen(g):
                for _ in g:
                    pass

            # ---------------- startup compute ----------------
            for tt in range(4):
                run_gen(gen_v(tt))
            run_gen(gen_kq(False, 0, 0))
            run_gen(gen_kq(True, 0, 0))

            # deadline-ordered filler queue (deadline = (mch, qc) at whose
            # start the group's output is first consumed)
            for m in range(NMC):
                for qc in range(NQC):
                    if m == 0 and qc == 0:
                        continue
                    fillers.append(((m, qc), gen_kq(False, m, qc)))
                    fillers.append(((m, qc), gen_kq(True, m, qc)))
                    if m == 0:
                        for tt in range(4 * qc, 4 * qc + 4):
                            fillers.append(((m, qc), gen_v(tt)))

            pending_norm = []

            def do_norm():
                while pending_norm:
                    m_, q_ = pending_norm.pop(0)
                    st = stage_sb[:, q_ % 4]
                    for j in range(2):
                        part = j * 64
                        rs = normp.tile([1, 512], f32, tag="rs")
                        nc.vector.reciprocal(rs[:], st[HD:HD + 1, j, :])
                        rep = normp.tile([64, 512], f32, tag="rep")
                        nc.gpsimd.partition_broadcast(rep[:], rs[0:1, :])
                        nc.vector.tensor_tensor(
                            AT_sb[part:part + 64, m_,
                                  q_ * 512:(q_ + 1) * 512],
                            st[0:HD, j, :], rep[:], mybir.AluOpType.mult,
                        )

            # ---------------- attention main loop ----------------
            for mch in range(NMC):
                for qc in range(NQC):
                    drain_until((mch, qc))
                    po = [ps_o.tile([HD + 1, 512], f32, tag="po",
                                    name=f"po{mch}_{qc}_{j}")
                          for j in range(2)]
                    nki = 4 * qc + 4
                    prev = None
                    for ki in range(nki):
                        off = max(0, ki - 4 * qc) * P
                        pshat = ps_st.tile([P, 2, 512], f32, tag="st")
                        pts = ptp.tile([P, 2, 512], bf16, tag="pt")
                        for j in range(2):
                            part = j * 64
                            nc.tensor.matmul(
                                pshat[:, j, off:512],
                                KT_sb[part:part + 64, mch,
                                      ki * P:(ki + 1) * P],
                                QT_sb[part:part + 64, mch,
                                      qc * 512 + off:(qc + 1) * 512],
                                start=True, stop=True,
                            )
                        nc.scalar.activation(
                            pts[:, :, off:512], pshat[:, :, off:512],
                            mybir.ActivationFunctionType.Exp,
                            scale=0.125,
                        )
                        if ki >= 4 * qc:
                            # diagonal block: zero out q < k entries
                            for j in range(2):
                                nc.vector.tensor_tensor(
                                    pts[:, j, off:off + P],
                                    pts[:, j, off:off + P],
                                    tri_sb[:], mybir.AluOpType.mult,
                                )
                        if ki == 1:
                            do_norm()
                        pump(3 if mch == NMC - 1 else 2)
                        if prev is not None:
                            poff, ppts = prev
                            for j in range(2):
                                nc.tensor.matmul(
                                    po[j][:, poff:512],
                                    V_sb[:, ki - 1, 2 * mch + j, :],
                                    ppts[:, j, poff:512],
                                    start=(ki - 1 == 0), stop=False,
                                )
                        prev = (off, pts)
                    poff, ppts = prev
                    for j in range(2):
                        nc.tensor.matmul(
                            po[j][:, poff:512],
                            V_sb[:, nki - 1, 2 * mch + j, :],
                            ppts[:, j, poff:512],
                            start=(nki == 1), stop=True,
                        )
                    if mch == NMC - 1 and qc == NQC - 1:
                        # last block: normalize straight from PSUM -- the
                        # staging copy would only lengthen the tail's
                        # critical chain
                        for j in range(2):
                            part = j * 64
                            rs = normp.tile([1, 512], f32, tag="rs")
                            nc.vector.reciprocal(rs[:], po[j][HD:HD + 1, :])
                            rep = normp.tile([64, 512], f32, tag="rep")
                            nc.gpsimd.partition_broadcast(rep[:], rs[0:1, :])
                            nc.vector.tensor_tensor(
                                AT_sb[part:part + 64, mch,
                                      qc * 512:(qc + 1) * 512],
                                po[j][0:HD, :], rep[:],
                                mybir.AluOpType.mult,
                            )
                    else:
                        # stage the unnormalized block out of PSUM (fast
                        # slot release); defer the normalize into the next
                        # qc's ki loop
                        st = stage_sb[:, qc % 4]
                        for j in range(2):
                            nc.vector.tensor_copy(st[0:HD + 1, j, :],
                                                  po[j][0:HD + 1, :])
                        pending_norm.append((mch, qc))
                    if mch == NMC - 1:
                        if qc < NQC - 1:
                            for tt in range(4 * qc, 4 * qc + 4):
                                for ncol in range(2):
                                    fillers.append(((9, qc),
                                                    gen_out(tt, ncol)))
            # ---- tail: last qc's output projection in two waves ----
            # wave 1 uses st/mm slots (free as soon as the last exp /
            # pumped copies retire) so its hc0-2 matmuls fill the PE while
            # the final normalize chain runs; wave 2 (po slots, freed by
            # that normalize) follows
            st_, po_, mm_ = (ps_st, "st"), (ps_o, "po"), (ps_mm, "mm")
            t0 = 4 * (NQC - 1)
            wave1 = [g for _, g in fillers] + [
                gen_out_pair(t0 + 0, st_, st_, nc.sync),
                gen_out_pair(t0 + 1, mm_, mm_, nc.scalar),
            ]
            wave2 = [
                gen_out_pair(t0 + 2, po_, po_, nc.sync),
                gen_out_pair(t0 + 3, st_, mm_, nc.scalar),
            ]
            for wave in (wave1, wave2):
                wave = deque(wave)
                while wave:
                    g = wave.popleft()
                    try:
                        next(g)
                    except StopIteration:
                        continue
                    wave.append(g)

    nc.compile()
    return nc


def host_inputs(x, w_qkv, b_qkv):
    """Per-core input maps. Core c -> batch c//2, head group c%2."""
    x = np.asarray(x, np.float32)
    w_qkv = np.asarray(w_qkv, np.float32)
    b_qkv = np.asarray(b_qkv, np.float32)
    tri = (np.arange(P)[None, :] >= np.arange(P)[:, None]).astype(BF16)
    in_maps = []
    for c in range(NCORES):
        b, g = c // 2, c % 2
        xT = np.ascontiguousarray(x[b].T).astype(BF16)
        # interleaved [K_m | Q_m] 128-col pairs
        wkq = np.empty((D, 2 * GCOLS), np.float32)
        bqk = np.empty((P, 2 * NMC), np.float32)
        for m in range(NMC):
            qs = g * GCOLS + m * P
            ks = D + g * GCOLS + m * P
            wkq[:, 256 * m:256 * m + P] = w_qkv[:, ks:ks + P]
            wkq[:, 256 * m + P:256 * (m + 1)] = w_qkv[:, qs:qs + P]
            bqk[:, 2 * m] = b_qkv[ks:ks + P]
            bqk[:, 2 * m + 1] = b_qkv[qs:qs + P]
        wkq = wkq.astype(BF16)
        wv = np.ascontiguousarray(
            w_qkv[:, 2 * D + g * GCOLS: 2 * D + (g + 1) * GCOLS]).astype(BF16)
        bv = np.ascontiguousarray(
            b_qkv[2 * D + g * GCOLS: 2 * D + (g + 1) * GCOLS]).astype(np.float32)
        in_maps.append({
            "xT": xT, "wkq": wkq, "wv": wv,
            "wp": None,  # filled by caller (needs w_proj)
            "bqk": bqk, "bv": bv, "tri": tri,
        })
    return in_maps


def full_in_maps(x, w_qkv, b_qkv, w_proj):
    w_proj = np.asarray(w_proj, np.float32)
    in_maps = host_inputs(x, w_qkv, b_qkv)
    for c in range(NCORES):
        g = c % 2
        in_maps[c]["wp"] = np.ascontiguousarray(
            w_proj[g * GCOLS:(g + 1) * GCOLS, :]).astype(BF16)
    return in_maps


def gather(results, b_proj):
    out = np.zeros((B, T, D), np.float32)
    for c in range(NCORES):
        out[c // 2] += results[c]["outp"].astype(np.float32)
    out += np.asarray(b_proj, np.float32)[None, None, :]
    return out


_NC_CACHE = None


def kernel(x, w_qkv, b_qkv, w_proj, b_proj):
    global _NC_CACHE
    if _NC_CACHE is None:
        _NC_CACHE = build_nc()
    in_maps = full_in_maps(x, w_qkv, b_qkv, w_proj)
    res = run_bass_kernel_spmd(_NC_CACHE, in_maps, core_ids=list(range(NCORES)))
    return gather(res.results, b_proj)


if __name__ == "__main__":
    rng = np.random.default_rng(0)
    x = rng.standard_normal((B, T, D), dtype=np.float32)
    w_qkv = rng.standard_normal((D, 3 * D), dtype=np.float32) / np.sqrt(D)
    b_qkv = np.zeros(3 * D, np.float32)
    w_proj = rng.standard_normal((D, D), dtype=np.float32) / np.sqrt(D)
    b_proj = np.zeros(D, np.float32)
    out = kernel(x, w_qkv, b_qkv, w_proj, b_proj)
    print(out.shape, out.dtype)
